# revision 16
# baseline (speedup 1.0000x reference)
"""Trainium2 Bass kernel for nn_CustomAttention (additive-tanh-score attention).

Math: out = softmax_m(mean_d tanh(q[n,d] + k[m,d])) @ v, with q = x1 Wq^T,
k = x2 Wk^T, v = x2 Wv^T.  The DropKey mask term (bernoulli * -1e-12) is below
fp32 resolution and is dropped.

Algorithm: tanh(s) is approximated by an odd-harmonic sine series
    tanh(s) ~= sum_i b_i sin(j_i * pi * s / L),   j_i = 1,3,...,19
so with theta_x = (pi/L) q_d, theta_y = (pi/L) k_d:
    sin(j(theta_x+theta_y)) = sin(j theta_x) cos(j theta_y)
                            + cos(j theta_x) sin(j theta_y)
which turns the [N,M,D] tanh reduction into a TensorE matmul with contraction
(2 * K * D).  Harmonic features sin/cos(j theta) are generated with the
three-term recurrence X_{j+2} = 2 cos(2 theta) X_j - X_{j-2} on the Vector
engine (ACT's Sin spline only covers [-pi, pi], so high harmonics cannot be
evaluated directly).  The series coefficients b_i are folded into the q-side
recurrence.  Softmax needs no max-subtraction (scores are means of tanh, so
|score| <= ~1) and the row-sum rides the output matmul as a ones-column of v.

Sharding: data-parallel over batch, 2 batches per core, 8 cores.
"""

import numpy as np

import concourse.bass as bass
import concourse.bacc as bacc
import concourse.mybir as mybir
from concourse.tile import TileContext
from concourse.bass_utils import run_bass_kernel_spmd

F32 = mybir.dt.float32
F32R = mybir.dt.float32r
AF = mybir.ActivationFunctionType
OP = mybir.AluOpType

# ---- fitted odd-harmonic sine series for tanh on |s| <= 6.96, L = half period
L_FIT = 11.504294395446777
B_COEF = [1.2350389628018632, 0.3265108349460186, 0.12969070001050748,
          0.054376297113699686, 0.022998492809357177, 0.009767106371444135,
          0.00412679540803737, 0.0017537431901711064, 0.0007544607820725653,
          0.0002955722082474476]
K = len(B_COEF)          # number of odd harmonics (1, 3, ..., 2K-1)

NCORES = 8
B_TOT, N, D = 16, 512, 64
BPC = B_TOT // NCORES    # batches per core
W = BPC * N              # free width when both batches are packed
PI = float(np.pi)

_cache = {}


def _build():
    """Build + compile the per-core Bass program (identical on all cores)."""
    nc = bacc.Bacc("TRN2", target_bir_lowering=False, debug=False)

    x1_d = nc.dram_tensor("x1", [BPC, N, D], F32, kind="ExternalInput")
    x2_d = nc.dram_tensor("x2", [BPC, N, D], F32, kind="ExternalInput")
    wq2_d = nc.dram_tensor("wq2", [D, 128], F32, kind="ExternalInput")
    wk2_d = nc.dram_tensor("wk2", [D, 128], F32, kind="ExternalInput")
    wv_d = nc.dram_tensor("wv", [D, D], F32, kind="ExternalInput")
    id_d = nc.dram_tensor("ident", [128, 128], F32, kind="ExternalInput")
    bq_d = nc.dram_tensor("biasq", [128, 1], F32, kind="ExternalInput")
    bk_d = nc.dram_tensor("biask", [128, 1], F32, kind="ExternalInput")
    cm2q_d = nc.dram_tensor("cm2q", [128, 2], F32, kind="ExternalInput")
    cm2k_d = nc.dram_tensor("cm2k", [128, 2], F32, kind="ExternalInput")
    out_d = nc.dram_tensor("out", [BPC, N, D], F32, kind="ExternalOutput")

    with TileContext(nc) as tc:
        with (
            tc.tile_pool(name="const", bufs=1) as const,
            tc.tile_pool(name="xin", bufs=1) as xin,
            tc.tile_pool(name="xt", bufs=2) as xt,
            tc.tile_pool(name="th", bufs=1) as thp,
            tc.tile_pool(name="mul", bufs=2) as mulp,
            tc.tile_pool(name="sqp", bufs=2) as sqp,
            tc.tile_pool(name="ladq", bufs=5) as ladq,
            tc.tile_pool(name="ladk", bufs=5) as ladk,
            tc.tile_pool(name="tmpq", bufs=2) as tmpq,
            tc.tile_pool(name="tmpk", bufs=2) as tmpk,
            tc.tile_pool(name="vaug", bufs=2) as vaugp,
            tc.tile_pool(name="ep", bufs=8) as ep,
            tc.tile_pool(name="osb", bufs=2) as osb,
            tc.tile_pool(name="rp", bufs=8) as rp,
            tc.tile_pool(name="ps", bufs=8, space="PSUM") as ps,
        ):
            # ---------- constants ----------
            sb_wq2 = const.tile([D, 128], F32)
            nc.sync.dma_start(out=sb_wq2, in_=wq2_d[:, :])
            sb_wk2 = const.tile([D, 128], F32)
            nc.sync.dma_start(out=sb_wk2, in_=wk2_d[:, :])
            sb_wv = const.tile([D, D], F32)
            nc.sync.dma_start(out=sb_wv, in_=wv_d[:, :])
            sb_id = const.tile([128, 128], F32)
            nc.sync.dma_start(out=sb_id, in_=id_d[:, :])
            sb_bq = const.tile([128, 1], F32)
            nc.sync.dma_start(out=sb_bq, in_=bq_d[:, :])
            sb_bk = const.tile([128, 1], F32)
            nc.sync.dma_start(out=sb_bk, in_=bk_d[:, :])
            sb_cm2q = const.tile([128, 2], F32)
            nc.sync.dma_start(out=sb_cm2q, in_=cm2q_d[:, :])
            sb_cm2k = const.tile([128, 2], F32)
            nc.sync.dma_start(out=sb_cm2k, in_=cm2k_d[:, :])

            # ---------- inputs ----------
            sb_x1 = xin.tile([128, BPC, 4, D], F32)
            sb_x2 = xin.tile([128, BPC, 4, D], F32)
            x1_r = x1_d.ap().rearrange("b (a p) d -> p b a d", p=128)
            x2_r = x2_d.ap().rearrange("b (a p) d -> p b a d", p=128)
            for b in range(BPC):
                nc.sync.dma_start(out=sb_x1[:, b], in_=x1_r[:, b])
                nc.sync.dma_start(out=sb_x2[:, b], in_=x2_r[:, b])

            # ---------- PE warm-up (HAM ramp): junk matmuls off the
            # critical path so transposes/projections run at full clock ----
            ps_junk = ps.tile([128, 128], F32, tag="bank", name="ps_junk")
            for w in range(12):
                nc.tensor.matmul(ps_junk, sb_id, sb_id, start=(w == 0),
                                 stop=(w == 11))

            # ---------- prologue: transposes, projections, v ----------
            sb_thq = thp.tile([128, W], F32)   # [sin-half d; cos-half d] x (b, n)
            sb_thk = thp.tile([128, W], F32)
            vaug = []
            for b in range(BPC):
                ps_x1t = ps.tile([D, N], F32, tag="bank")
                ps_x2t = ps.tile([D, N], F32, tag="bank")
                for a in range(4):
                    nc.tensor.transpose(
                        ps_x1t[:, a * 128:(a + 1) * 128], sb_x1[:, b, a, :], sb_id)
                    nc.tensor.transpose(
                        ps_x2t[:, a * 128:(a + 1) * 128], sb_x2[:, b, a, :], sb_id)
                sb_x1t = xt.tile([D, N], F32)
                nc.vector.tensor_copy(sb_x1t, ps_x1t)
                sb_x2t = xt.tile([D, N], F32)
                nc.vector.tensor_copy(sb_x2t, ps_x2t)

                ps_thq = ps.tile([128, N], F32, tag="bank")
                nc.tensor.matmul(ps_thq, sb_wq2, sb_x1t, start=True, stop=True)
                nc.vector.tensor_copy(sb_thq[:, b * N:(b + 1) * N], ps_thq)
                ps_thk = ps.tile([128, N], F32, tag="bank")
                nc.tensor.matmul(ps_thk, sb_wk2, sb_x2t, start=True, stop=True)
                nc.vector.tensor_copy(sb_thk[:, b * N:(b + 1) * N], ps_thk)

                ps_v = ps.tile([128, 4, D], F32, tag="bank")
                for a in range(4):
                    nc.tensor.matmul(
                        ps_v[:, a, :], sb_x2t[:, a * 128:(a + 1) * 128], sb_wv,
                        start=True, stop=True)
                sb_va = vaugp.tile([128, 4, D + 1], F32)
                nc.vector.memset(sb_va, 1.0)
                nc.vector.tensor_copy(sb_va[:, :, 0:D], ps_v)
                vaug.append(sb_va)

            # ---------- harmonic bases ----------
            # q side: X_i = b-scaled [sin((2i+1)th); cos((2i+1)th)]
            # k side: Z_i =          [cos((2i+1)th); sin((2i+1)th)]
            # z1/x1b/xs1 first: they alone gate the first score matmuls.
            z1 = ladk.tile([128, W], F32, tag="ladk")       # [cos th; sin th]
            nc.scalar.activation(z1, sb_thk, AF.Sin, bias=sb_bk[:, 0:1], scale=1.0)
            x1b = ladq.tile([128, W], F32, tag="ladq")      # [sin th; cos th]
            nc.scalar.activation(x1b, sb_thq, AF.Sin, bias=sb_bq[:, 0:1], scale=1.0)
            xs1 = ladq.tile([128, W], F32, tag="ladq")
            nc.vector.tensor_scalar(xs1, x1b, float(B_COEF[0]), None, OP.mult)

            xm1 = ladq.tile([128, W], F32, tag="ladq")      # j = -1: [-sin th; cos th]
            nc.scalar.activation(xm1, sb_thq, AF.Sin, bias=sb_bq[:, 0:1], scale=-1.0)
            zm1 = ladk.tile([128, W], F32, tag="ladk")      # j = -1: [cos th; -sin th]
            nc.scalar.activation(zm1, sb_thk, AF.Sin, bias=sb_bk[:, 0:1], scale=-1.0)

            # multipliers cos(2 th) (q) / 2cos(2 th) (k) from Square of bases
            sq_q = sqp.tile([128, W], F32, tag="sq", name="sq_q")
            nc.scalar.activation(sq_q, x1b, AF.Square, bias=0.0, scale=1.0)
            m2q = mulp.tile([128, W], F32, name="m2q")
            nc.vector.tensor_scalar(
                m2q, sq_q, sb_cm2q[:, 0:1], sb_cm2q[:, 1:2], OP.mult, OP.add)
            sq_k = sqp.tile([128, W], F32, tag="sq", name="sq_k")
            nc.scalar.activation(sq_k, z1, AF.Square, bias=0.0, scale=1.0)
            m2k = mulp.tile([128, W], F32, name="m2k")
            nc.vector.tensor_scalar(
                m2k, sq_k, sb_cm2k[:, 0:1], sb_cm2k[:, 1:2], OP.mult, OP.add)

            # prefetch the exp table set while the ladder runs (ACT idle)
            sb_warm = sqp.tile([1, 1], F32, tag="warm", name="sb_warm")
            nc.scalar.activation(sb_warm, m2q[0:1, 0:1], AF.Exp, bias=0.0,
                                 scale=1.0)

            # ---------- scores psum ----------
            ps_sc = [[ps.tile([128, N], F32, tag="bank", name=f"ps_sc_{b}_{mt}")
                      for mt in range(4)] for b in range(BPC)]

            xq_prev, xq_cur = xm1, xs1
            zk_prev, zk_cur = zm1, z1
            for i in range(1, K):
                # q side, b-folded (DVE):
                rm = 2.0 * B_COEF[i] / B_COEF[i - 1]
                rs = B_COEF[i] / (1.0 if i == 1 else B_COEF[i - 2])
                tq = tmpq.tile([128, W], F32)
                nc.vector.scalar_tensor_tensor(
                    tq, xq_cur, float(rm), m2q, OP.mult, OP.mult)
                xq_new = ladq.tile([128, W], F32R, tag="ladq", name="xq_new")
                nc.vector.scalar_tensor_tensor(
                    xq_new, xq_prev, float(-rs), tq, OP.mult, OP.add)
                xq_prev, xq_cur = xq_cur, xq_new
                # k side, unscaled (m2k holds 2cos2th), gpsimd head / DVE tail
                tk = tmpk.tile([128, W], F32)
                zk_new = ladk.tile([128, W], F32R, tag="ladk", name="zk_new")
                if i <= 5:
                    nc.gpsimd.tensor_mul(tk, zk_cur, m2k)
                    nc.gpsimd.tensor_sub(zk_new, tk, zk_prev)
                else:
                    nc.vector.tensor_mul(tk, zk_cur, m2k)
                    nc.vector.tensor_sub(zk_new, tk, zk_prev)
                zk_prev, zk_cur = zk_cur, zk_new
                # harmonic i score matmuls (fp32r, fast path)
                for b in range(BPC):
                    for mt in range(4):
                        nc.tensor.matmul(
                            ps_sc[b][mt],
                            zk_new[:, b * N + mt * 128: b * N + (mt + 1) * 128],
                            xq_new[:, b * N:(b + 1) * N],
                            start=(i == 1), stop=False)

            # the big fp32 j=1 term last: PE is fully warm, and psum
            # accumulation order is free.
            for b in range(BPC):
                for mt in range(4):
                    nc.tensor.matmul(
                        ps_sc[b][mt],
                        z1[:, b * N + mt * 128: b * N + (mt + 1) * 128],
                        xs1[:, b * N:(b + 1) * N],
                        start=False, stop=True)

            # ---------- epilogue: softmax (no max-sub) + output ----------
            for b in range(BPC):
                e_tiles = []
                for mt in range(4):
                    e = ep.tile([128, N], F32)
                    nc.scalar.activation(
                        e, ps_sc[b][mt], AF.Exp, bias=0.0, scale=1.0 / D)
                    e_tiles.append(e)
                o_sb = osb.tile([128, 4, D], F32)
                for nt in range(4):
                    ps_on = ps.tile([128, D + 1], F32, tag="bank",
                                    name=f"ps_on_{b}_{nt}")
                    for mt in range(4):
                        nc.tensor.matmul(
                            ps_on, e_tiles[mt][:, nt * 128:(nt + 1) * 128],
                            vaug[b][:, mt, :], start=(mt == 0), stop=(mt == 3))
                    r = rp.tile([128, 1], F32)
                    nc.vector.reciprocal(r, ps_on[:, D:D + 1])
                    nc.vector.tensor_scalar(
                        o_sb[:, nt, :], ps_on[:, 0:D], r[:, 0:1], None, OP.mult)
                nc.sync.dma_start(
                    out=out_d.ap().rearrange("b (a p) d -> p b a d", p=128)[:, b],
                    in_=o_sb)

    nc.compile()
    return nc


def _host_prep(Wq, Wk, Wv):
    scale = np.float32(np.pi / L_FIT)
    wq2 = np.concatenate([(scale * Wq).T, (scale * Wq).T], axis=1).astype(np.float32)
    wk2 = np.concatenate([(scale * Wk).T, (scale * Wk).T], axis=1).astype(np.float32)
    wv = np.ascontiguousarray(Wv.T.astype(np.float32))
    ident = np.eye(128, dtype=np.float32)
    biasq = np.concatenate([np.zeros(64), np.full(64, np.pi / 2)]).astype(
        np.float32).reshape(128, 1)
    biask = np.concatenate([np.full(64, np.pi / 2), np.zeros(64)]).astype(
        np.float32).reshape(128, 1)
    cm2q = np.stack([np.concatenate([np.full(64, -2.0), np.full(64, 2.0)]),
                     np.concatenate([np.full(64, 1.0), np.full(64, -1.0)])],
                    axis=1).astype(np.float32)
    cm2k = np.stack([np.concatenate([np.full(64, 4.0), np.full(64, -4.0)]),
                     np.concatenate([np.full(64, -2.0), np.full(64, 2.0)])],
                    axis=1).astype(np.float32)
    return wq2, wk2, wv, ident, biasq, biask, cm2q, cm2k


def kernel(input1, input2, Wq, Wk, Wv):
    if "nc" not in _cache:
        _cache["nc"] = _build()
    nc = _cache["nc"]

    (wq2, wk2, wv, ident, biasq, biask, cm2q, cm2k) = _host_prep(
        np.asarray(Wq), np.asarray(Wk), np.asarray(Wv))
    x1 = np.ascontiguousarray(np.asarray(input1, dtype=np.float32))
    x2 = np.ascontiguousarray(np.asarray(input2, dtype=np.float32))

    in_maps = []
    for c in range(NCORES):
        in_maps.append({
            "x1": x1[c * BPC:(c + 1) * BPC],
            "x2": x2[c * BPC:(c + 1) * BPC],
            "wq2": wq2, "wk2": wk2, "wv": wv,
            "ident": ident, "biasq": biasq, "biask": biask,
            "cm2q": cm2q, "cm2k": cm2k,
        })
    res = run_bass_kernel_spmd(nc, in_maps, core_ids=list(range(NCORES)))
    out = np.concatenate([res.results[c]["out"] for c in range(NCORES)], axis=0)
    return out.astype(np.float32)


# revision 17
# speedup vs baseline: 1.1189x; 1.1189x over previous
"""Trainium2 Bass kernel for nn_CustomAttention (additive-tanh-score attention).

Math: out = softmax_m(mean_d tanh(q[n,d] + k[m,d])) @ v, with q = x1 Wq^T,
k = x2 Wk^T, v = x2 Wv^T.  The DropKey mask term (bernoulli * -1e-12) is below
fp32 resolution and is dropped.

Algorithm: tanh(s) is approximated by an odd-harmonic sine series
    tanh(s) ~= sum_i b_i sin(j_i * pi * s / L),   j_i = 1,3,...,19
so with theta_x = (pi/L) q_d, theta_y = (pi/L) k_d:
    sin(j(theta_x+theta_y)) = sin(j theta_x) cos(j theta_y)
                            + cos(j theta_x) sin(j theta_y)
which turns the [N,M,D] tanh reduction into a TensorE matmul with contraction
(2 * K * D).  Harmonic features sin/cos(j theta) are generated with the
three-term recurrence X_{j+2} = 2 cos(2 theta) X_j - X_{j-2} on the Vector
engine (ACT's Sin spline only covers [-pi, pi], so high harmonics cannot be
evaluated directly).  The series coefficients b_i are folded into the q-side
recurrence.  Softmax needs no max-subtraction (scores are means of tanh, so
|score| <= ~1) and the row-sum rides the output matmul as a ones-column of v.

Sharding: data-parallel over batch, 2 batches per core, 8 cores.
"""

import numpy as np

import concourse.bass as bass
import concourse.bacc as bacc
import concourse.mybir as mybir
from concourse.tile import TileContext
from concourse.bass_utils import run_bass_kernel_spmd

F32 = mybir.dt.float32
F32R = mybir.dt.float32r
AF = mybir.ActivationFunctionType
OP = mybir.AluOpType

# ---- fitted odd-harmonic sine series for tanh on |s| <= 6.96, L = half period
L_FIT = 11.504294395446777
B_COEF = [1.2350389628018632, 0.3265108349460186, 0.12969070001050748,
          0.054376297113699686, 0.022998492809357177, 0.009767106371444135,
          0.00412679540803737, 0.0017537431901711064, 0.0007544607820725653,
          0.0002955722082474476]
K = len(B_COEF)          # number of odd harmonics (1, 3, ..., 2K-1)

NCORES = 8
B_TOT, N, D = 16, 512, 64
BPC = B_TOT // NCORES    # batches per core
W = BPC * N              # free width when both batches are packed
PI = float(np.pi)

_cache = {}


def _build():
    """Build + compile the per-core Bass program (identical on all cores)."""
    nc = bacc.Bacc("TRN2", target_bir_lowering=False, debug=False)

    x1_d = nc.dram_tensor("x1", [BPC, N, D], F32, kind="ExternalInput")
    x2_d = nc.dram_tensor("x2", [BPC, N, D], F32, kind="ExternalInput")
    wq2_d = nc.dram_tensor("wq2", [D, 128], F32, kind="ExternalInput")
    wk2_d = nc.dram_tensor("wk2", [D, 128], F32, kind="ExternalInput")
    wv_d = nc.dram_tensor("wv", [D, D], F32, kind="ExternalInput")
    id_d = nc.dram_tensor("ident", [128, 128], F32, kind="ExternalInput")
    bq_d = nc.dram_tensor("biasq", [128, 1], F32, kind="ExternalInput")
    bk_d = nc.dram_tensor("biask", [128, 1], F32, kind="ExternalInput")
    cm2q_d = nc.dram_tensor("cm2q", [128, 2], F32, kind="ExternalInput")
    cm2k_d = nc.dram_tensor("cm2k", [128, 2], F32, kind="ExternalInput")
    out_d = nc.dram_tensor("out", [BPC, N, D], F32, kind="ExternalOutput")

    with TileContext(nc) as tc:
        with (
            tc.tile_pool(name="const", bufs=1) as const,
            tc.tile_pool(name="xin", bufs=1) as xin,
            tc.tile_pool(name="xt", bufs=2) as xt,
            tc.tile_pool(name="th", bufs=1) as thp,
            tc.tile_pool(name="mul", bufs=2) as mulp,
            tc.tile_pool(name="sqp", bufs=2) as sqp,
            tc.tile_pool(name="ladq", bufs=5) as ladq,
            tc.tile_pool(name="ladk", bufs=5) as ladk,
            tc.tile_pool(name="tmpq", bufs=2) as tmpq,
            tc.tile_pool(name="tmpk", bufs=2) as tmpk,
            tc.tile_pool(name="vaug", bufs=2) as vaugp,
            tc.tile_pool(name="ep", bufs=8) as ep,
            tc.tile_pool(name="osb", bufs=2) as osb,
            tc.tile_pool(name="rp", bufs=8) as rp,
            tc.tile_pool(name="ps", bufs=8, space="PSUM") as ps,
        ):
            # ---------- constants ----------
            sb_wq2 = const.tile([D, 128], F32)
            nc.sync.dma_start(out=sb_wq2, in_=wq2_d[:, :])
            sb_wk2 = const.tile([D, 128], F32)
            nc.sync.dma_start(out=sb_wk2, in_=wk2_d[:, :])
            sb_wv = const.tile([D, D], F32)
            nc.sync.dma_start(out=sb_wv, in_=wv_d[:, :])
            sb_id = const.tile([128, 128], F32)
            nc.sync.dma_start(out=sb_id, in_=id_d[:, :])
            sb_bq = const.tile([128, 1], F32)
            nc.sync.dma_start(out=sb_bq, in_=bq_d[:, :])
            sb_bk = const.tile([128, 1], F32)
            nc.sync.dma_start(out=sb_bk, in_=bk_d[:, :])
            sb_cm2q = const.tile([128, 2], F32)
            nc.sync.dma_start(out=sb_cm2q, in_=cm2q_d[:, :])
            sb_cm2k = const.tile([128, 2], F32)
            nc.sync.dma_start(out=sb_cm2k, in_=cm2k_d[:, :])

            # ---------- inputs ----------
            sb_x1 = xin.tile([128, BPC, 4, D], F32)
            sb_x2 = xin.tile([128, BPC, 4, D], F32)
            x1_r = x1_d.ap().rearrange("b (a p) d -> p b a d", p=128)
            x2_r = x2_d.ap().rearrange("b (a p) d -> p b a d", p=128)
            for b in range(BPC):
                nc.sync.dma_start(out=sb_x1[:, b], in_=x1_r[:, b])
                nc.sync.dma_start(out=sb_x2[:, b], in_=x2_r[:, b])

            # ---------- PE warm-up (HAM ramp): junk matmuls off the
            # critical path so transposes/projections run at full clock ----
            ps_junk = ps.tile([128, 128], F32, tag="bank", name="ps_junk")
            for w in range(6):
                nc.tensor.matmul(ps_junk, sb_id, sb_id, start=(w == 0),
                                 stop=(w == 5))

            # ---------- prologue: transposes, projections, v ----------
            sb_thq = thp.tile([128, W], F32)   # [sin-half d; cos-half d] x (b, n)
            sb_thk = thp.tile([128, W], F32)
            vaug = []
            for b in range(BPC):
                ps_x1t = ps.tile([D, N], F32, tag="bank")
                ps_x2t = ps.tile([D, N], F32, tag="bank")
                for a in range(4):
                    nc.tensor.transpose(
                        ps_x1t[:, a * 128:(a + 1) * 128], sb_x1[:, b, a, :], sb_id)
                    nc.tensor.transpose(
                        ps_x2t[:, a * 128:(a + 1) * 128], sb_x2[:, b, a, :], sb_id)
                sb_x1t = xt.tile([D, N], F32)
                nc.vector.tensor_copy(sb_x1t, ps_x1t)
                sb_x2t = xt.tile([D, N], F32)
                nc.vector.tensor_copy(sb_x2t, ps_x2t)

                ps_thq = ps.tile([128, N], F32, tag="bank")
                nc.tensor.matmul(ps_thq, sb_wq2, sb_x1t, start=True, stop=True)
                nc.vector.tensor_copy(sb_thq[:, b * N:(b + 1) * N], ps_thq)
                ps_thk = ps.tile([128, N], F32, tag="bank")
                nc.tensor.matmul(ps_thk, sb_wk2, sb_x2t, start=True, stop=True)
                nc.vector.tensor_copy(sb_thk[:, b * N:(b + 1) * N], ps_thk)

                ps_v = ps.tile([128, 4, D], F32, tag="bank")
                for a in range(4):
                    nc.tensor.matmul(
                        ps_v[:, a, :], sb_x2t[:, a * 128:(a + 1) * 128], sb_wv,
                        start=True, stop=True)
                sb_va = vaugp.tile([128, 4, D + 1], F32)
                nc.vector.memset(sb_va, 1.0)
                nc.vector.tensor_copy(sb_va[:, :, 0:D], ps_v)
                vaug.append(sb_va)

            # ---------- harmonic bases ----------
            # q side: X_i = b-scaled [sin((2i+1)th); cos((2i+1)th)]
            # k side: Z_i =          [cos((2i+1)th); sin((2i+1)th)]
            # z1/x1b/xs1 first: they alone gate the first score matmuls.
            z1 = ladk.tile([128, W], F32, tag="ladk")       # [cos th; sin th]
            nc.scalar.activation(z1, sb_thk, AF.Sin, bias=sb_bk[:, 0:1], scale=1.0)
            x1b = ladq.tile([128, W], F32, tag="ladq")      # [sin th; cos th]
            nc.scalar.activation(x1b, sb_thq, AF.Sin, bias=sb_bq[:, 0:1], scale=1.0)
            xs1 = ladq.tile([128, W], F32, tag="ladq")
            nc.vector.tensor_scalar(xs1, x1b, float(B_COEF[0]), None, OP.mult)

            xm1 = ladq.tile([128, W], F32, tag="ladq")      # j = -1: [-sin th; cos th]
            nc.scalar.activation(xm1, sb_thq, AF.Sin, bias=sb_bq[:, 0:1], scale=-1.0)
            zm1 = ladk.tile([128, W], F32, tag="ladk")      # j = -1: [cos th; -sin th]
            nc.scalar.activation(zm1, sb_thk, AF.Sin, bias=sb_bk[:, 0:1], scale=-1.0)

            # multipliers cos(2 th) (q) / 2cos(2 th) (k) from Square of bases
            sq_q = sqp.tile([128, W], F32, tag="sq", name="sq_q")
            nc.scalar.activation(sq_q, x1b, AF.Square, bias=0.0, scale=1.0)
            m2q = mulp.tile([128, W], F32, name="m2q")
            nc.vector.tensor_scalar(
                m2q, sq_q, sb_cm2q[:, 0:1], sb_cm2q[:, 1:2], OP.mult, OP.add)
            sq_k = sqp.tile([128, W], F32, tag="sq", name="sq_k")
            nc.scalar.activation(sq_k, z1, AF.Square, bias=0.0, scale=1.0)
            m2k = mulp.tile([128, W], F32, name="m2k")
            nc.vector.tensor_scalar(
                m2k, sq_k, sb_cm2k[:, 0:1], sb_cm2k[:, 1:2], OP.mult, OP.add)

            # prefetch the exp table set while the ladder runs (ACT idle)
            sb_warm = sqp.tile([1, 1], F32, tag="warm", name="sb_warm")
            nc.scalar.activation(sb_warm, m2q[0:1, 0:1], AF.Exp, bias=0.0,
                                 scale=1.0)

            # ---------- scores psum ----------
            ps_sc = [[ps.tile([128, N], F32, tag="bank", name=f"ps_sc_{b}_{mt}")
                      for mt in range(4)] for b in range(BPC)]

            # i = 0 score matmuls (fp32, exact j=1 term) gate only on xs1/z1
            for b in range(BPC):
                for mt in range(4):
                    nc.tensor.matmul(
                        ps_sc[b][mt],
                        z1[:, b * N + mt * 128: b * N + (mt + 1) * 128],
                        xs1[:, b * N:(b + 1) * N],
                        start=True, stop=False)

            xq_prev, xq_cur = xm1, xs1
            zk_prev, zk_cur = zm1, z1
            for i in range(1, K):
                # q side, b-folded (DVE)
                rm = 2.0 * B_COEF[i] / B_COEF[i - 1]
                rs = B_COEF[i] / (1.0 if i == 1 else B_COEF[i - 2])
                tq = tmpq.tile([128, W], F32)
                nc.vector.scalar_tensor_tensor(
                    tq, xq_cur, float(rm), m2q, OP.mult, OP.mult)
                xq_new = ladq.tile([128, W], F32R, tag="ladq", name="xq_new")
                nc.vector.scalar_tensor_tensor(
                    xq_new, xq_prev, float(-rs), tq, OP.mult, OP.add)
                xq_prev, xq_cur = xq_cur, xq_new
                # k side, unscaled (m2k holds 2cos2th): gpsimd head, DVE tail
                tk = tmpk.tile([128, W], F32)
                zk_new = ladk.tile([128, W], F32R, tag="ladk", name="zk_new")
                if i <= 5:
                    nc.gpsimd.tensor_mul(tk, zk_cur, m2k)
                    nc.gpsimd.tensor_sub(zk_new, tk, zk_prev)
                else:
                    nc.vector.tensor_mul(tk, zk_cur, m2k)
                    nc.vector.tensor_sub(zk_new, tk, zk_prev)
                zk_prev, zk_cur = zk_cur, zk_new
                # harmonic i score matmuls (fp32r fast path)
                for b in range(BPC):
                    for mt in range(4):
                        nc.tensor.matmul(
                            ps_sc[b][mt],
                            zk_new[:, b * N + mt * 128: b * N + (mt + 1) * 128],
                            xq_new[:, b * N:(b + 1) * N],
                            start=False, stop=(i == K - 1))

            # ---------- epilogue: softmax (no max-sub) + output ----------
            for b in range(BPC):
                e_tiles = []
                for mt in range(4):
                    e = ep.tile([128, N], F32)
                    nc.scalar.activation(
                        e, ps_sc[b][mt], AF.Exp, bias=0.0, scale=1.0 / D)
                    e_tiles.append(e)
                o_sb = osb.tile([128, 4, D], F32)
                for nt in range(4):
                    ps_on = ps.tile([128, D + 1], F32, tag="bank",
                                    name=f"ps_on_{b}_{nt}")
                    for mt in range(4):
                        nc.tensor.matmul(
                            ps_on, e_tiles[mt][:, nt * 128:(nt + 1) * 128],
                            vaug[b][:, mt, :], start=(mt == 0), stop=(mt == 3))
                    r = rp.tile([128, 1], F32)
                    nc.vector.reciprocal(r, ps_on[:, D:D + 1])
                    nc.vector.tensor_scalar(
                        o_sb[:, nt, :], ps_on[:, 0:D], r[:, 0:1], None, OP.mult)
                nc.sync.dma_start(
                    out=out_d.ap().rearrange("b (a p) d -> p b a d", p=128)[:, b],
                    in_=o_sb)

    nc.compile()
    return nc


def _host_prep(Wq, Wk, Wv):
    scale = np.float32(np.pi / L_FIT)
    wq2 = np.concatenate([(scale * Wq).T, (scale * Wq).T], axis=1).astype(np.float32)
    wk2 = np.concatenate([(scale * Wk).T, (scale * Wk).T], axis=1).astype(np.float32)
    wv = np.ascontiguousarray(Wv.T.astype(np.float32))
    ident = np.eye(128, dtype=np.float32)
    biasq = np.concatenate([np.zeros(64), np.full(64, np.pi / 2)]).astype(
        np.float32).reshape(128, 1)
    biask = np.concatenate([np.full(64, np.pi / 2), np.zeros(64)]).astype(
        np.float32).reshape(128, 1)
    cm2q = np.stack([np.concatenate([np.full(64, -2.0), np.full(64, 2.0)]),
                     np.concatenate([np.full(64, 1.0), np.full(64, -1.0)])],
                    axis=1).astype(np.float32)
    cm2k = np.stack([np.concatenate([np.full(64, 4.0), np.full(64, -4.0)]),
                     np.concatenate([np.full(64, -2.0), np.full(64, 2.0)])],
                    axis=1).astype(np.float32)
    return wq2, wk2, wv, ident, biasq, biask, cm2q, cm2k


def kernel(input1, input2, Wq, Wk, Wv):
    if "nc" not in _cache:
        _cache["nc"] = _build()
    nc = _cache["nc"]

    (wq2, wk2, wv, ident, biasq, biask, cm2q, cm2k) = _host_prep(
        np.asarray(Wq), np.asarray(Wk), np.asarray(Wv))
    x1 = np.ascontiguousarray(np.asarray(input1, dtype=np.float32))
    x2 = np.ascontiguousarray(np.asarray(input2, dtype=np.float32))

    in_maps = []
    for c in range(NCORES):
        in_maps.append({
            "x1": x1[c * BPC:(c + 1) * BPC],
            "x2": x2[c * BPC:(c + 1) * BPC],
            "wq2": wq2, "wk2": wk2, "wv": wv,
            "ident": ident, "biasq": biasq, "biask": biask,
            "cm2q": cm2q, "cm2k": cm2k,
        })
    res = run_bass_kernel_spmd(nc, in_maps, core_ids=list(range(NCORES)))
    out = np.concatenate([res.results[c]["out"] for c in range(NCORES)], axis=0)
    return out.astype(np.float32)


# revision 18
# speedup vs baseline: 1.1923x; 1.0656x over previous
"""Trainium2 Bass kernel for nn_CustomAttention (additive-tanh-score attention).

Math: out = softmax_m(mean_d tanh(q[n,d] + k[m,d])) @ v, with q = x1 Wq^T,
k = x2 Wk^T, v = x2 Wv^T.  The DropKey mask term (bernoulli * -1e-12) is below
fp32 resolution and is dropped.

Algorithm: tanh(s) is approximated by an odd-harmonic sine series
    tanh(s) ~= sum_i b_i sin(j_i * pi * s / L),   j_i = 1,3,...,19
so with theta_x = (pi/L) q_d, theta_y = (pi/L) k_d:
    sin(j(theta_x+theta_y)) = sin(j theta_x) cos(j theta_y)
                            + cos(j theta_x) sin(j theta_y)
which turns the [N,M,D] tanh reduction into a TensorE matmul with contraction
(2 * K * D).  Harmonic features sin/cos(j theta) are generated with the
three-term recurrence X_{j+2} = 2 cos(2 theta) X_j - X_{j-2} on the Vector
engine (ACT's Sin spline only covers [-pi, pi], so high harmonics cannot be
evaluated directly).  The series coefficients b_i are folded into the q-side
recurrence.  Softmax needs no max-subtraction (scores are means of tanh, so
|score| <= ~1) and the row-sum rides the output matmul as a ones-column of v.

Sharding: data-parallel over batch, 2 batches per core, 8 cores.
"""

import numpy as np

import concourse.bass as bass
import concourse.bacc as bacc
import concourse.mybir as mybir
from concourse.tile import TileContext
from concourse.bass_utils import run_bass_kernel_spmd

F32 = mybir.dt.float32
F32R = mybir.dt.float32r
AF = mybir.ActivationFunctionType
OP = mybir.AluOpType

# ---- fitted odd-harmonic sine series for tanh on |s| <= 6.96, L = half period
L_FIT = 11.504294395446777
B_COEF = [1.2350389628018632, 0.3265108349460186, 0.12969070001050748,
          0.054376297113699686, 0.022998492809357177, 0.009767106371444135,
          0.00412679540803737, 0.0017537431901711064, 0.0007544607820725653,
          0.0002955722082474476]
K = len(B_COEF)          # number of odd harmonics (1, 3, ..., 2K-1)

NCORES = 8
B_TOT, N, D = 16, 512, 64
BPC = B_TOT // NCORES    # batches per core
W = BPC * N              # free width when both batches are packed
PI = float(np.pi)

_cache = {}


def _build():
    """Build + compile the per-core Bass program (identical on all cores)."""
    nc = bacc.Bacc("TRN2", target_bir_lowering=False, debug=False)

    x1_d = nc.dram_tensor("x1", [BPC, N, D], F32, kind="ExternalInput")
    x2_d = nc.dram_tensor("x2", [BPC, N, D], F32, kind="ExternalInput")
    wq2_d = nc.dram_tensor("wq2", [D, 128], F32, kind="ExternalInput")
    wk2_d = nc.dram_tensor("wk2", [D, 128], F32, kind="ExternalInput")
    wv_d = nc.dram_tensor("wv", [D, D], F32, kind="ExternalInput")
    id_d = nc.dram_tensor("ident", [128, 128], F32, kind="ExternalInput")
    bq_d = nc.dram_tensor("biasq", [128, 1], F32, kind="ExternalInput")
    bk_d = nc.dram_tensor("biask", [128, 1], F32, kind="ExternalInput")
    cm2q_d = nc.dram_tensor("cm2q", [128, 2], F32, kind="ExternalInput")
    cm2k_d = nc.dram_tensor("cm2k", [128, 2], F32, kind="ExternalInput")
    pmk_d = nc.dram_tensor("pmk", [128, 1], F32, kind="ExternalInput")
    out_d = nc.dram_tensor("out", [BPC, N, D], F32, kind="ExternalOutput")

    with TileContext(nc) as tc:
        with (
            tc.tile_pool(name="const", bufs=1) as const,
            tc.tile_pool(name="xin", bufs=1) as xin,
            tc.tile_pool(name="xt", bufs=2) as xt,
            tc.tile_pool(name="th", bufs=1) as thp,
            tc.tile_pool(name="mul", bufs=2) as mulp,
            tc.tile_pool(name="sqp", bufs=2) as sqp,
            tc.tile_pool(name="ladq", bufs=5) as ladq,
            tc.tile_pool(name="ladk", bufs=10) as ladk,
            tc.tile_pool(name="tmpq", bufs=2) as tmpq,
            tc.tile_pool(name="tmpk", bufs=3) as tmpk,
            tc.tile_pool(name="vaug", bufs=2) as vaugp,
            tc.tile_pool(name="ep", bufs=8) as ep,
            tc.tile_pool(name="osb", bufs=2) as osb,
            tc.tile_pool(name="rp", bufs=8) as rp,
            tc.tile_pool(name="ps", bufs=8, space="PSUM") as ps,
        ):
            # ---------- constants ----------
            sb_wq2 = const.tile([D, 128], F32)
            nc.sync.dma_start(out=sb_wq2, in_=wq2_d[:, :])
            sb_wk2 = const.tile([D, 128], F32)
            nc.sync.dma_start(out=sb_wk2, in_=wk2_d[:, :])
            sb_wv = const.tile([D, D], F32)
            nc.sync.dma_start(out=sb_wv, in_=wv_d[:, :])
            sb_id = const.tile([128, 128], F32)
            nc.sync.dma_start(out=sb_id, in_=id_d[:, :])
            sb_bq = const.tile([128, 1], F32)
            nc.sync.dma_start(out=sb_bq, in_=bq_d[:, :])
            sb_bk = const.tile([128, 1], F32)
            nc.sync.dma_start(out=sb_bk, in_=bk_d[:, :])
            sb_cm2q = const.tile([128, 2], F32)
            nc.sync.dma_start(out=sb_cm2q, in_=cm2q_d[:, :])
            sb_cm2k = const.tile([128, 2], F32)
            nc.sync.dma_start(out=sb_cm2k, in_=cm2k_d[:, :])
            sb_pmk = const.tile([128, 1], F32)
            nc.sync.dma_start(out=sb_pmk, in_=pmk_d[:, :])

            # ---------- inputs ----------
            sb_x1 = xin.tile([128, BPC, 4, D], F32)
            sb_x2 = xin.tile([128, BPC, 4, D], F32)
            x1_r = x1_d.ap().rearrange("b (p a) d -> p b a d", a=4)
            x2_r = x2_d.ap().rearrange("b (p a) d -> p b a d", a=4)
            for b in range(BPC):
                nc.sync.dma_start(out=sb_x1[:, b], in_=x1_r[:, b])
                nc.sync.dma_start(out=sb_x2[:, b], in_=x2_r[:, b])

            # ---------- PE warm-up (HAM ramp): junk matmuls off the
            # critical path so transposes/projections run at full clock ----
            ps_junk = ps.tile([128, 128], F32, tag="bank", name="ps_junk")
            for w in range(6):
                nc.tensor.matmul(ps_junk, sb_id, sb_id, start=(w == 0),
                                 stop=(w == 5))

            # ---------- prologue: transposes, projections, v ----------
            sb_thq = thp.tile([128, W], F32)   # [sin-half d; cos-half d] x (b, n)
            sb_thk = thp.tile([128, W], F32)
            vaug = []
            for b in range(BPC):
                ps_x1t = ps.tile([D, N], F32, tag="bank")
                ps_x2t = ps.tile([D, N], F32, tag="bank")
                for a in range(4):
                    nc.tensor.transpose(
                        ps_x1t[:, a * 128:(a + 1) * 128], sb_x1[:, b, a, :], sb_id)
                    nc.tensor.transpose(
                        ps_x2t[:, a * 128:(a + 1) * 128], sb_x2[:, b, a, :], sb_id)
                sb_x1t = xt.tile([D, N], F32)
                nc.vector.tensor_copy(sb_x1t, ps_x1t)
                sb_x2t = xt.tile([D, N], F32)
                nc.vector.tensor_copy(sb_x2t, ps_x2t)

                ps_thq = ps.tile([128, N], F32, tag="bank")
                nc.tensor.matmul(ps_thq, sb_wq2, sb_x1t, start=True, stop=True)
                nc.vector.tensor_copy(sb_thq[:, b * N:(b + 1) * N], ps_thq)
                ps_thk = ps.tile([128, N], F32, tag="bank")
                nc.tensor.matmul(ps_thk, sb_wk2, sb_x2t, start=True, stop=True)
                nc.vector.tensor_copy(sb_thk[:, b * N:(b + 1) * N], ps_thk)

                ps_v = ps.tile([128, 4, D], F32, tag="bank")
                for a in range(4):
                    nc.tensor.matmul(
                        ps_v[:, a, :], sb_x2t[:, a * 128:(a + 1) * 128], sb_wv,
                        start=True, stop=True)
                sb_va = vaugp.tile([128, 4, D + 1], F32)
                nc.vector.memset(sb_va, 1.0)
                nc.vector.tensor_copy(sb_va[:, :, 0:D], ps_v)
                vaug.append(sb_va)

            # ---------- harmonic bases ----------
            # q side: X_i = b-scaled [sin((2i+1)th); cos((2i+1)th)]
            # k side: Z_i =          [cos((2i+1)th); sin((2i+1)th)]
            # z1/x1b/xs1 first: they alone gate the first score matmuls.
            z1 = ladk.tile([128, W], F32, tag="ladk")       # [cos th; sin th]
            nc.scalar.activation(z1, sb_thk, AF.Sin, bias=sb_bk[:, 0:1], scale=1.0)
            x1b = ladq.tile([128, W], F32, tag="ladq")      # [sin th; cos th]
            nc.scalar.activation(x1b, sb_thq, AF.Sin, bias=sb_bq[:, 0:1], scale=1.0)
            xs1 = ladq.tile([128, W], F32, tag="ladq")
            nc.vector.tensor_scalar(xs1, x1b, float(B_COEF[0]), None, OP.mult)

            xm1 = ladq.tile([128, W], F32, tag="ladq")      # j = -1: [-sin th; cos th]
            nc.scalar.activation(xm1, sb_thq, AF.Sin, bias=sb_bq[:, 0:1], scale=-1.0)
            zm1 = ladk.tile([128, W], F32, tag="ladk")      # j = -1: [cos th; -sin th]
            nc.scalar.activation(zm1, sb_thk, AF.Sin, bias=sb_bk[:, 0:1], scale=-1.0)

            # multipliers cos(2 th) (q) / 2cos(2 th) (k) from Square of bases
            sq_q = sqp.tile([128, W], F32, tag="sq", name="sq_q")
            nc.scalar.activation(sq_q, x1b, AF.Square, bias=0.0, scale=1.0)
            m2q = mulp.tile([128, W], F32, name="m2q")
            nc.vector.tensor_scalar(
                m2q, sq_q, sb_cm2q[:, 0:1], sb_cm2q[:, 1:2], OP.mult, OP.add)
            sq_k = sqp.tile([128, W], F32, tag="sq", name="sq_k")
            nc.scalar.activation(sq_k, z1, AF.Square, bias=0.0, scale=1.0)
            m2k = mulp.tile([128, W], F32, name="m2k")
            nc.vector.tensor_scalar(
                m2k, sq_k, sb_cm2k[:, 0:1], sb_cm2k[:, 1:2], OP.mult, OP.add)

            # prefetch the exp table set while the ladder runs (ACT idle)
            sb_warm = sqp.tile([1, 1], F32, tag="warm", name="sb_warm")
            nc.scalar.activation(sb_warm, m2q[0:1, 0:1], AF.Exp, bias=0.0,
                                 scale=1.0)

            # ---------- scores psum ----------
            ps_sc = [[ps.tile([128, N], F32, tag="bank", name=f"ps_sc_{b}_{mt}")
                      for mt in range(4)] for b in range(BPC)]

            # i = 0 score matmuls (fp32, exact j=1 term) gate only on xs1/z1
            for b in range(BPC):
                for mt in range(4):
                    nc.tensor.matmul(
                        ps_sc[b][mt],
                        z1[:, b * N + mt * 128: b * N + (mt + 1) * 128],
                        xs1[:, b * N:(b + 1) * N],
                        start=True, stop=False)

            # ---- k-side step-4 sub-chains (multiplier M4 = m2k^2 - 2) ----
            sqm4 = sqp.tile([128, W], F32, tag="sq", name="sqm4")
            nc.scalar.activation(sqm4, m2k, AF.Square, bias=0.0, scale=1.0)
            m4k = mulp.tile([128, W], F32, name="m4k")
            nc.vector.tensor_scalar(m4k, sqm4, -2.0, None, OP.add)
            # Z_3 (harmonic index 1) via one step-2 ladder step on DVE
            tk3 = tmpk.tile([128, W], F32, name="tk3")
            nc.vector.tensor_mul(tk3, z1, m2k)
            z3 = ladk.tile([128, W], F32R, tag="ladk", name="z3")
            nc.vector.tensor_sub(z3, tk3, zm1)
            # Z_-3 = flip of Z_3 (bottom half negated), on ACT
            zb3 = ladk.tile([128, W], F32, tag="ladk", name="zb3")
            nc.scalar.activation(zb3, z3, AF.Identity, bias=0.0,
                                 scale=sb_pmk[:, 0:1])

            # k-even chain {5, 9, 13, 17} on gpsimd; k-odd {7, 11, 15, 19}
            # split gpsimd/DVE.  zh[i] = tile for harmonic 2i+1.
            zh = [z1, z3] + [None] * (K - 2)
            ze_prev, ze_cur = zb3, z1
            zo_prev, zo_cur = zm1, z3
            for step in range(4):
                # even: harmonic idx 2 + 2*step
                te = tmpk.tile([128, W], F32, name="te")
                nc.gpsimd.tensor_mul(te, ze_cur, m4k)
                ze_new = ladk.tile([128, W], F32R, tag="ladk", name="ze_new")
                nc.gpsimd.tensor_sub(ze_new, te, ze_prev)
                ze_prev, ze_cur = ze_cur, ze_new
                zh[2 + 2 * step] = ze_new
                # odd: harmonic idx 3 + 2*step
                to = tmpk.tile([128, W], F32, name="to")
                zo_new = ladk.tile([128, W], F32R, tag="ladk", name="zo_new")
                if step < 2:
                    nc.gpsimd.tensor_mul(to, zo_cur, m4k)
                    nc.gpsimd.tensor_sub(zo_new, to, zo_prev)
                else:
                    nc.vector.tensor_mul(to, zo_cur, m4k)
                    nc.vector.tensor_sub(zo_new, to, zo_prev)
                zo_prev, zo_cur = zo_cur, zo_new
                zh[3 + 2 * step] = zo_new

            # ---- q-side b-folded chain (DVE) + score matmuls as they land --
            xq_prev, xq_cur = xm1, xs1
            for i in range(1, K):
                rm = 2.0 * B_COEF[i] / B_COEF[i - 1]
                rs = B_COEF[i] / (1.0 if i == 1 else B_COEF[i - 2])
                tq = tmpq.tile([128, W], F32)
                nc.vector.scalar_tensor_tensor(
                    tq, xq_cur, float(rm), m2q, OP.mult, OP.mult)
                xq_new = ladq.tile([128, W], F32R, tag="ladq", name="xq_new")
                nc.vector.scalar_tensor_tensor(
                    xq_new, xq_prev, float(-rs), tq, OP.mult, OP.add)
                xq_prev, xq_cur = xq_cur, xq_new
                for b in range(BPC):
                    for mt in range(4):
                        nc.tensor.matmul(
                            ps_sc[b][mt],
                            zh[i][:, b * N + mt * 128: b * N + (mt + 1) * 128],
                            xq_new[:, b * N:(b + 1) * N],
                            start=False, stop=(i == K - 1))

            # ---------- epilogue: softmax (no max-sub) + output ----------
            for b in range(BPC):
                e_tiles = []
                for mt in range(4):
                    e = ep.tile([128, N], F32)
                    nc.scalar.activation(
                        e, ps_sc[b][mt], AF.Exp, bias=0.0, scale=1.0 / D)
                    e_tiles.append(e)
                o_sb = osb.tile([128, 4, D], F32)
                for nt in range(4):
                    ps_on = ps.tile([128, D + 1], F32, tag="bank",
                                    name=f"ps_on_{b}_{nt}")
                    for mt in range(4):
                        nc.tensor.matmul(
                            ps_on, e_tiles[mt][:, nt * 128:(nt + 1) * 128],
                            vaug[b][:, mt, :], start=(mt == 0), stop=(mt == 3))
                    r = rp.tile([128, 1], F32)
                    nc.vector.reciprocal(r, ps_on[:, D:D + 1])
                    nc.vector.tensor_scalar(
                        o_sb[:, nt, :], ps_on[:, 0:D], r[:, 0:1], None, OP.mult)
                nc.sync.dma_start(
                    out=out_d.ap().rearrange("b (p a) d -> p b a d", a=4)[:, b],
                    in_=o_sb)

    nc.compile()
    return nc


def _host_prep(Wq, Wk, Wv):
    scale = np.float32(np.pi / L_FIT)
    wq2 = np.concatenate([(scale * Wq).T, (scale * Wq).T], axis=1).astype(np.float32)
    wk2 = np.concatenate([(scale * Wk).T, (scale * Wk).T], axis=1).astype(np.float32)
    wv = np.ascontiguousarray(Wv.T.astype(np.float32))
    ident = np.eye(128, dtype=np.float32)
    biasq = np.concatenate([np.zeros(64), np.full(64, np.pi / 2)]).astype(
        np.float32).reshape(128, 1)
    biask = np.concatenate([np.full(64, np.pi / 2), np.zeros(64)]).astype(
        np.float32).reshape(128, 1)
    cm2q = np.stack([np.concatenate([np.full(64, -2.0), np.full(64, 2.0)]),
                     np.concatenate([np.full(64, 1.0), np.full(64, -1.0)])],
                    axis=1).astype(np.float32)
    cm2k = np.stack([np.concatenate([np.full(64, 4.0), np.full(64, -4.0)]),
                     np.concatenate([np.full(64, -2.0), np.full(64, 2.0)])],
                    axis=1).astype(np.float32)
    pmk = np.concatenate([np.full(64, 1.0), np.full(64, -1.0)]).astype(
        np.float32).reshape(128, 1)
    return wq2, wk2, wv, ident, biasq, biask, cm2q, cm2k, pmk


def kernel(input1, input2, Wq, Wk, Wv):
    if "nc" not in _cache:
        _cache["nc"] = _build()
    nc = _cache["nc"]

    (wq2, wk2, wv, ident, biasq, biask, cm2q, cm2k, pmk) = _host_prep(
        np.asarray(Wq), np.asarray(Wk), np.asarray(Wv))
    x1 = np.ascontiguousarray(np.asarray(input1, dtype=np.float32))
    x2 = np.ascontiguousarray(np.asarray(input2, dtype=np.float32))

    in_maps = []
    for c in range(NCORES):
        in_maps.append({
            "x1": x1[c * BPC:(c + 1) * BPC],
            "x2": x2[c * BPC:(c + 1) * BPC],
            "wq2": wq2, "wk2": wk2, "wv": wv,
            "ident": ident, "biasq": biasq, "biask": biask,
            "cm2q": cm2q, "cm2k": cm2k, "pmk": pmk,
        })
    res = run_bass_kernel_spmd(nc, in_maps, core_ids=list(range(NCORES)))
    out = np.concatenate([res.results[c]["out"] for c in range(NCORES)], axis=0)
    return out.astype(np.float32)
